# revision 1
# baseline (speedup 1.0000x reference)
"""AttentiveStatsPooling Trainium2 kernel.

Full-input contract: kernel(**inputs) takes the unsharded numpy inputs
  x            (32, 1536, 2048) f32
  padding_mask (32, 2048)       bool
  W_tdnn       (128, 1536)      f32
  b_tdnn       (128,)           f32
  W_attn       (1536, 128)      f32
  b_attn       (1536,)          f32
and returns the full (32, 3072) f32 output.

Sharding: data-parallel over batch. 8 cores x 4 samples each, weights
replicated. Math per sample:
  e    = tanh(W_tdnn @ x + b_tdnn)            (BN, T)
  a    = W_attn @ e  (+ b_attn: dropped - constant along T, cancels in
                      the softmax over T)      (C, T)
  a   += -1e9 * mask[t]                        (additive mask; exp -> 0)
  S0   = sum_t exp(a);  S1 = sum_t exp(a)*x;  S2 = sum_t exp(a)*x^2
  mean = S1/S0;  std = sqrt(clip(S2/S0 - mean^2, 1e-9))
All matmuls/products in bf16 with fp32 accumulation (PSUM / reduce
accumulators): HW-verified relative error 2.1e-4 (scale-rel absmax 7.5e-4).

Performance (measured on HW, ~300-330 us/core; 1.7x over the naive
schedule). Engine assignment chosen from on-HW microbenchmarks:
  - any DVE op with accum_out runs 1x (fast perf-modes disabled by the
    accumulator), so reductions cost ~2.2us/[128,2048] everywhere;
  - products (tensor_tensor bf16) do hit the 2x mode (1.17us);
  - exp on ACT reads logits straight from PSUM, its accumulator gives
    S0 for free; S1 reduces on DVE, S2 on ACT (Copy+accum) to balance
    both engines at ~235us busy;
  - the S2 stage is pipelined 2 steps behind, and two samples' chunk
    streams are interleaved so each engine fills the other stream's
    cross-engine dependency bubbles (the single biggest win).
"""

import numpy as np
import ml_dtypes

B, C, T = 32, 1536, 2048
BN = 128
NCORES = 8
SPC = B // NCORES  # samples per core
CK = C // 128      # c chunks of 128 partitions
NJ = T // 512      # 512-wide column groups (one PSUM bank each)

BF16 = ml_dtypes.bfloat16

_PROG_CACHE = {}


def _build_program(reps=None):
    """Build the per-core program. reps=None: straight-line body.
    reps=K: wrap the whole body in a hardware For_i loop (timing only)."""
    import concourse.bacc as bacc
    import concourse.tile as tile
    import concourse.mybir as mybir
    from contextlib import nullcontext
    from concourse.bass_interp import get_hw_module

    dt = mybir.dt
    AF = mybir.ActivationFunctionType
    OP = mybir.AluOpType

    nc = bacc.Bacc(
        "TRN2",
        target_bir_lowering=False,
        debug=False,
        num_devices=NCORES,
        num_swdge_queues=4,
    )
    x_d = nc.dram_tensor("x", [SPC, C, T], dt.bfloat16, kind="ExternalInput")
    mn_d = nc.dram_tensor("maskneg", [SPC, T], dt.bfloat16, kind="ExternalInput")
    wt_d = nc.dram_tensor("wt", [C, BN], dt.bfloat16, kind="ExternalInput")
    wa_d = nc.dram_tensor("wa", [BN, C], dt.bfloat16, kind="ExternalInput")
    bt_d = nc.dram_tensor("bt", [BN, 1], dt.float32, kind="ExternalInput")
    out_d = nc.dram_tensor("out", [SPC, 2 * C], dt.float32, kind="ExternalOutput")

    with tile.TileContext(nc) as tc:
        with (
            tc.tile_pool(name="const", bufs=1) as constp,
            tc.tile_pool(name="xin", bufs=2 * CK) as xp,
            tc.tile_pool(name="esb", bufs=3) as ep,
            tc.tile_pool(name="expm", bufs=3) as xpm,
            tc.tile_pool(name="prod", bufs=4) as prp,
            tc.tile_pool(name="mneg", bufs=2) as mnp,
            tc.tile_pool(name="s0p", bufs=4) as s0pp,
            tc.tile_pool(name="stats", bufs=1) as statsp,
            tc.tile_pool(name="tail", bufs=2) as tailp,
            tc.tile_pool(name="ps", bufs=2, space="PSUM") as psp,
        ):
            # ---- constants ------------------------------------------------
            wt_sb = constp.tile([128, CK, BN], dt.bfloat16, tag="wt")
            nc.sync.dma_start(
                out=wt_sb, in_=wt_d.ap().rearrange("(k p) o -> p k o", p=128)
            )
            wa_sb = constp.tile([128, C], dt.bfloat16, tag="wa")
            nc.sync.dma_start(out=wa_sb, in_=wa_d.ap())
            bt_sb = constp.tile([128, 1], dt.float32, tag="bt")
            nc.sync.dma_start(out=bt_sb, in_=bt_d.ap())
            ones_sb = constp.tile([1, 128], dt.bfloat16, tag="ones")
            nc.vector.memset(ones_sb, 1.0)

            loop_cm = tc.For_i(0, reps, 1) if reps is not None else nullcontext()
            with loop_cm:
                stats = []
                for s in range(SPC):
                    S0 = statsp.tile([128, CK], dt.float32, tag=f"S0_{s}")
                    S1 = statsp.tile([128, CK], dt.float32, tag=f"S1_{s}")
                    S2 = statsp.tile([128, CK], dt.float32, tag=f"S2_{s}")
                    stats.append((S0, S1, S2))

                # process samples in pairs; the two chunk streams interleave
                # so ACT/DVE always have an independent chunk to work on
                for s0 in range(0, SPC, 2):
                    pair = [s0, s0 + 1]
                    xts = {}
                    mnegs = {}
                    esbs = {}
                    for s in pair:
                        mneg_sb = mnp.tile(
                            [1, T], dt.bfloat16, tag="mneg", name=f"mneg_{s}"
                        )
                        nc.sync.dma_start(out=mneg_sb, in_=mn_d.ap()[s : s + 1, :])
                        mnegs[s] = mneg_sb
                        for k in range(CK):
                            xt = xp.tile(
                                [128, T], dt.bfloat16, tag="x", name=f"x_{s}_{k}"
                            )
                            nc.sync.dma_start(
                                out=xt, in_=x_d.ap()[s, k * 128 : (k + 1) * 128, :]
                            )
                            xts[(s, k)] = xt

                    # mm1 + tanh for both samples of the pair
                    for s in pair:
                        pse = psp.tile(
                            [128, T], dt.float32, tag="ps", name=f"pse_{s}"
                        )
                        for j in range(NJ):
                            for k in range(CK):
                                nc.tensor.matmul(
                                    pse[:, j * 512 : (j + 1) * 512],
                                    lhsT=wt_sb[:, k, :],
                                    rhs=xts[(s, k)][:, j * 512 : (j + 1) * 512],
                                    start=(k == 0),
                                    stop=(k == CK - 1),
                                )
                        e_sb = ep.tile([128, T], dt.bfloat16, tag="e", name=f"e_{s}")
                        nc.scalar.activation(
                            out=e_sb, in_=pse, func=AF.Tanh, bias=bt_sb, scale=1.0
                        )
                        esbs[s] = e_sb

                    def s2_stage(s, c, p2):
                        if (s * CK + c) % 16 == 0:
                            nc.vector.tensor_reduce(
                                out=stats[s][2][:, c : c + 1],
                                in_=p2,
                                op=OP.add,
                                axis=mybir.AxisListType.X,
                            )
                        else:
                            junk = prp.tile(
                                [128, T], dt.bfloat16, tag="junk",
                                name=f"junk_{s}_{c}",
                            )
                            nc.scalar.activation(
                                out=junk,
                                in_=p2,
                                func=AF.Copy,
                                accum_out=stats[s][2][:, c : c + 1],
                            )

                    pending = []
                    for c in range(CK):
                        for s in pair:
                            S0, S1, S2 = stats[s]
                            e_sb = esbs[s]
                            mneg_sb = mnegs[s]
                            expm = xpm.tile(
                                [128, T], dt.bfloat16, tag="expm",
                                name=f"expm_{s}_{c}",
                            )
                            pa = psp.tile(
                                [128, T], dt.float32, tag="ps", name=f"pa_{s}_{c}"
                            )
                            for jj in range(NJ):
                                nc.tensor.matmul(
                                    pa[:, jj * 512 : (jj + 1) * 512],
                                    lhsT=wa_sb[:, c * 128 : (c + 1) * 128],
                                    rhs=e_sb[:, jj * 512 : (jj + 1) * 512],
                                    start=True,
                                    stop=False,
                                )
                            for jj in range(NJ):
                                nc.tensor.matmul(
                                    pa[:, jj * 512 : (jj + 1) * 512],
                                    lhsT=ones_sb[:, :],
                                    rhs=mneg_sb[:, jj * 512 : (jj + 1) * 512],
                                    start=False,
                                    stop=True,
                                )
                            nc.scalar.activation(
                                out=expm,
                                in_=pa,
                                func=AF.Exp,
                                accum_out=S0[:, c : c + 1],
                            )
                            p1 = prp.tile(
                                [128, T], dt.bfloat16, tag="p1", name=f"p1_{s}_{c}"
                            )
                            nc.vector.tensor_tensor(
                                out=p1, in0=expm, in1=xts[(s, c)], op=OP.mult
                            )
                            p2 = prp.tile(
                                [128, T], dt.bfloat16, tag="p2", name=f"p2_{s}_{c}"
                            )
                            nc.vector.tensor_tensor(
                                out=p2, in0=p1, in1=xts[(s, c)], op=OP.mult
                            )
                            nc.vector.tensor_reduce(
                                out=S1[:, c : c + 1],
                                in_=p1,
                                op=OP.add,
                                axis=mybir.AxisListType.X,
                            )
                            pending.append((s, c, p2))
                            if len(pending) > 2:
                                s2_stage(*pending.pop(0))
                    for item in pending:
                        s2_stage(*item)

                # ---- tail: mean/std + output DMA --------------------------
                for s in range(SPC):
                    S0, S1, S2 = stats[s]
                    r0 = tailp.tile([128, CK], dt.float32, tag="r0", name=f"r0_{s}")
                    nc.vector.reciprocal(out=r0, in_=S0)
                    mean = tailp.tile(
                        [128, CK], dt.float32, tag="mean", name=f"mean_{s}"
                    )
                    nc.vector.tensor_tensor(out=mean, in0=S1, in1=r0, op=OP.mult)
                    ex2 = tailp.tile([128, CK], dt.float32, tag="ex2", name=f"ex2_{s}")
                    nc.vector.tensor_tensor(out=ex2, in0=S2, in1=r0, op=OP.mult)
                    m2 = tailp.tile([128, CK], dt.float32, tag="m2", name=f"m2_{s}")
                    nc.vector.tensor_tensor(out=m2, in0=mean, in1=mean, op=OP.mult)
                    var = tailp.tile([128, CK], dt.float32, tag="var", name=f"var_{s}")
                    nc.vector.tensor_tensor(out=var, in0=ex2, in1=m2, op=OP.subtract)
                    nc.vector.tensor_scalar(
                        out=var,
                        in0=var,
                        scalar1=1e-9,
                        scalar2=None,
                        op0=OP.max,
                    )
                    std = tailp.tile([128, CK], dt.float32, tag="std", name=f"std_{s}")
                    nc.scalar.activation(out=std, in_=var, func=AF.Sqrt)
                    nc.sync.dma_start(
                        out=out_d.ap()[s, 0:C].rearrange("(ck p) -> p ck", p=128),
                        in_=mean,
                    )
                    nc.sync.dma_start(
                        out=out_d.ap()[s, C : 2 * C].rearrange(
                            "(ck p) -> p ck", p=128
                        ),
                        in_=std,
                    )

    nc.compile()
    nc.m = get_hw_module(nc.m)
    return nc


def _get_program():
    if "nc" not in _PROG_CACHE:
        _PROG_CACHE["nc"] = _build_program()
    return _PROG_CACHE["nc"]


def _prep_inputs(x, padding_mask, W_tdnn, b_tdnn, W_attn, b_attn):
    """Host-side prep: cast/transpose, build per-core input maps."""
    xb = np.ascontiguousarray(x).astype(BF16)
    maskneg = np.where(padding_mask, np.float32(-1e9), np.float32(0.0)).astype(BF16)
    wt = np.ascontiguousarray(W_tdnn.T).astype(BF16)  # (C, BN)
    wa = np.ascontiguousarray(W_attn.T).astype(BF16)  # (BN, C)
    bt = np.ascontiguousarray(b_tdnn.astype(np.float32).reshape(BN, 1))
    in_maps = []
    for i in range(NCORES):
        sl = slice(i * SPC, (i + 1) * SPC)
        in_maps.append(
            {
                "x": np.ascontiguousarray(xb[sl]),
                "maskneg": np.ascontiguousarray(maskneg[sl]),
                "wt": wt,
                "wa": wa,
                "bt": bt,
            }
        )
    return in_maps


def kernel(x, padding_mask, W_tdnn, b_tdnn, W_attn, b_attn):
    from concourse.bass_utils import run_bass_kernel_spmd

    nc = _get_program()
    in_maps = _prep_inputs(x, padding_mask, W_tdnn, b_tdnn, W_attn, b_attn)
    res = run_bass_kernel_spmd(nc, in_maps, core_ids=list(range(NCORES)))
    out = np.concatenate([res.results[i]["out"] for i in range(NCORES)], axis=0)
    return out.astype(np.float32)



# revision 5
# speedup vs baseline: 1.3773x; 1.3773x over previous
"""AttentiveStatsPooling Trainium2 kernel.

Full-input contract: kernel(**inputs) takes the unsharded numpy inputs
  x            (32, 1536, 2048) f32
  padding_mask (32, 2048)       bool
  W_tdnn       (128, 1536)      f32
  b_tdnn       (128,)           f32
  W_attn       (1536, 128)      f32
  b_attn       (1536,)          f32
and returns the full (32, 3072) f32 output.

Sharding: data-parallel over batch. 8 cores x 4 samples each, weights
replicated. Math per sample:
  e    = tanh(W_tdnn @ x + b_tdnn)            (BN, T)
  a    = W_attn @ e  (+ b_attn: dropped - constant along T, cancels in
                      the softmax over T)      (C, T)
  a   += -1e9 * mask[t]                        (additive mask; exp -> 0)
  S0   = sum_t exp(a);  S1 = sum_t exp(a)*x;  S2 = sum_t exp(a)*x^2
  mean = S1/S0;  std = sqrt(clip(S2/S0 - mean^2, 1e-9))

Key optimizations over the naive schedule:
  1. Host-side mask compaction: masked columns contribute exactly 0 to
     all three sums (exp(-1e9) == 0), and the sums are permutation-
     invariant, so only the unmasked columns of x are shipped/processed,
     zero-padded per-sample to a common width Tp (~56% of T for the
     ~50% random mask). Every per-element op and the x DMA shrink
     proportionally. Pad columns still get -1e9 via the mask row.
  2. Fused product+reduce: scalar_tensor_tensor computes
     out = in0*in1, accum_out = sum(out) in ONE 1x-rate DVE op, so S1
     is free with the p1 product. S2 likewise on a fraction of chunks;
     the remaining chunks compute p2 = p1*x on DVE (2x rate) and reduce
     on ACT (Copy+accum_out), balancing ACT and DVE busy time.
  3. exp reads logits straight from PSUM; its accumulator gives S0 free.
All matmuls/products in bf16 with fp32 accumulation.
"""

import numpy as np
import ml_dtypes

B, C, T = 32, 1536, 2048
BN = 128
NCORES = 8
SPC = B // NCORES  # samples per core
CK = C // 128      # c chunks of 128 partitions

BF16 = ml_dtypes.bfloat16

# Default compacted width; _prep_inputs overrides from the actual mask.
_STATE = {"tp": 1152}

# Chunks (of CK=12) whose S2 reduction runs on ACT (Copy+accum);
# the rest fuse into a second DVE tensor_tensor_reduce.
S2_ACT_CHUNKS = (0, 2, 4, 6, 8, 10, 11)

_PROG_CACHE = {}


def _build_program(reps=None, tp=None):
    """Build the per-core program. reps=None: straight-line body.
    reps=K: wrap the whole body in a hardware For_i loop (timing only)."""
    import concourse.bacc as bacc
    import concourse.tile as tile
    import concourse.mybir as mybir
    from contextlib import nullcontext
    from concourse.bass_interp import get_hw_module

    dt = mybir.dt
    AF = mybir.ActivationFunctionType
    OP = mybir.AluOpType

    TP = int(tp if tp is not None else _STATE["tp"])
    # PSUM bank groups (free-dim tiles of <=512 fp32); PSUM tiles are
    # allocated at a whole-bank width so every pool slot stays
    # bank-aligned (matmul outputs must not straddle banks).
    banks = [(j, min(j + 512, TP)) for j in range(0, TP, 512)]
    PSW = -(-TP // 512) * 512

    nc = bacc.Bacc(
        "TRN2",
        target_bir_lowering=False,
        debug=False,
        num_devices=NCORES,
        num_swdge_queues=4,
    )
    x_d = nc.dram_tensor("x", [SPC, C, TP], dt.bfloat16, kind="ExternalInput")
    mn_d = nc.dram_tensor("maskneg", [SPC, TP], dt.bfloat16, kind="ExternalInput")
    wt_d = nc.dram_tensor("wt", [C, BN], dt.bfloat16, kind="ExternalInput")
    wa_d = nc.dram_tensor("wa", [BN, C], dt.bfloat16, kind="ExternalInput")
    bt_d = nc.dram_tensor("bt", [BN, 1], dt.float32, kind="ExternalInput")
    out_d = nc.dram_tensor("out", [SPC, 2 * C], dt.float32, kind="ExternalOutput")

    with tile.TileContext(nc) as tc:
        with (
            tc.tile_pool(name="const", bufs=1) as constp,
            tc.tile_pool(name="xin", bufs=2 * CK) as xp,
            tc.tile_pool(name="esb", bufs=3) as ep,
            tc.tile_pool(name="expm", bufs=3) as xpm,
            tc.tile_pool(name="prod", bufs=6) as prp,
            tc.tile_pool(name="mneg", bufs=2) as mnp,
            tc.tile_pool(name="stats", bufs=1) as statsp,
            tc.tile_pool(name="tail", bufs=2) as tailp,
            tc.tile_pool(name="ps", bufs=2, space="PSUM") as psp,
        ):
            # ---- constants ------------------------------------------------
            wt_sb = constp.tile([128, CK, BN], dt.bfloat16, tag="wt")
            nc.sync.dma_start(
                out=wt_sb, in_=wt_d.ap().rearrange("(k p) o -> p k o", p=128)
            )
            wa_sb = constp.tile([128, C], dt.bfloat16, tag="wa")
            nc.sync.dma_start(out=wa_sb, in_=wa_d.ap())
            bt_sb = constp.tile([128, 1], dt.float32, tag="bt")
            nc.sync.dma_start(out=bt_sb, in_=bt_d.ap())
            ones_sb = constp.tile([1, 128], dt.bfloat16, tag="ones")
            nc.vector.memset(ones_sb, 1.0)

            loop_cm = tc.For_i(0, reps, 1) if reps is not None else nullcontext()
            with loop_cm:
                stats = []
                for s in range(SPC):
                    S0 = statsp.tile([128, CK], dt.float32, tag=f"S0_{s}")
                    S1 = statsp.tile([128, CK], dt.float32, tag=f"S1_{s}")
                    S2 = statsp.tile([128, CK], dt.float32, tag=f"S2_{s}")
                    stats.append((S0, S1, S2))

                def tail_stage(s):
                    # mean/std + output DMA for one finished sample
                    S0, S1, S2 = stats[s]
                    r0 = tailp.tile([128, CK], dt.float32, tag="r0", name=f"r0_{s}")
                    nc.vector.reciprocal(out=r0, in_=S0)
                    mean = tailp.tile(
                        [128, CK], dt.float32, tag="mean", name=f"mean_{s}"
                    )
                    nc.vector.tensor_tensor(out=mean, in0=S1, in1=r0, op=OP.mult)
                    ex2 = tailp.tile(
                        [128, CK], dt.float32, tag="ex2", name=f"ex2_{s}"
                    )
                    nc.vector.tensor_tensor(out=ex2, in0=S2, in1=r0, op=OP.mult)
                    m2 = tailp.tile([128, CK], dt.float32, tag="m2", name=f"m2_{s}")
                    nc.vector.tensor_tensor(out=m2, in0=mean, in1=mean, op=OP.mult)
                    var = tailp.tile(
                        [128, CK], dt.float32, tag="var", name=f"var_{s}"
                    )
                    nc.vector.tensor_tensor(out=var, in0=ex2, in1=m2, op=OP.subtract)
                    nc.vector.tensor_scalar(
                        out=var, in0=var, scalar1=1e-9, scalar2=None, op0=OP.max
                    )
                    std = tailp.tile(
                        [128, CK], dt.float32, tag="std", name=f"std_{s}"
                    )
                    nc.scalar.activation(out=std, in_=var, func=AF.Sqrt)
                    nc.sync.dma_start(
                        out=out_d.ap()[s, 0:C].rearrange("(ck p) -> p ck", p=128),
                        in_=mean,
                    )
                    nc.sync.dma_start(
                        out=out_d.ap()[s, C : 2 * C].rearrange(
                            "(ck p) -> p ck", p=128
                        ),
                        in_=std,
                    )

                # process samples in pairs; the two chunk streams interleave
                # so ACT/DVE always have an independent chunk to work on
                for s0 in range(0, SPC, 2):
                    pair = [s0, s0 + 1]
                    xts = {}
                    mnegs = {}
                    esbs = {}
                    for s in pair:
                        mneg_sb = mnp.tile(
                            [1, TP], dt.bfloat16, tag="mneg", name=f"mneg_{s}"
                        )
                        nc.sync.dma_start(out=mneg_sb, in_=mn_d.ap()[s : s + 1, :])
                        mnegs[s] = mneg_sb
                        for k in range(CK):
                            xt = xp.tile(
                                [128, TP], dt.bfloat16, tag="x", name=f"x_{s}_{k}"
                            )
                            nc.sync.dma_start(
                                out=xt, in_=x_d.ap()[s, k * 128 : (k + 1) * 128, :]
                            )
                            xts[(s, k)] = xt

                    # mm1 + tanh for both samples of the pair
                    for s in pair:
                        pse_full = psp.tile(
                            [128, PSW], dt.float32, tag="ps", name=f"pse_{s}"
                        )
                        pse = pse_full[:, :TP]
                        for (j0, j1) in banks:
                            for k in range(CK):
                                nc.tensor.matmul(
                                    pse[:, j0:j1],
                                    lhsT=wt_sb[:, k, :],
                                    rhs=xts[(s, k)][:, j0:j1],
                                    start=(k == 0),
                                    stop=(k == CK - 1),
                                )
                        e_sb = ep.tile([128, TP], dt.bfloat16, tag="e", name=f"e_{s}")
                        nc.scalar.activation(
                            out=e_sb, in_=pse, func=AF.Tanh, bias=bt_sb, scale=1.0
                        )
                        esbs[s] = e_sb

                    # prior pair's tails overlap with this pair's chunk work
                    if s0 > 0:
                        tail_stage(s0 - 2)
                        tail_stage(s0 - 1)

                    for c in range(CK):
                        for s in pair:
                            S0, S1, S2 = stats[s]
                            e_sb = esbs[s]
                            mneg_sb = mnegs[s]
                            xt = xts[(s, c)]
                            expm = xpm.tile(
                                [128, TP], dt.bfloat16, tag="expm",
                                name=f"expm_{s}_{c}",
                            )
                            pa_full = psp.tile(
                                [128, PSW], dt.float32, tag="ps",
                                name=f"pa_{s}_{c}",
                            )
                            pa = pa_full[:, :TP]
                            for (j0, j1) in banks:
                                nc.tensor.matmul(
                                    pa[:, j0:j1],
                                    lhsT=wa_sb[:, c * 128 : (c + 1) * 128],
                                    rhs=e_sb[:, j0:j1],
                                    start=True,
                                    stop=False,
                                )
                            for (j0, j1) in banks:
                                nc.tensor.matmul(
                                    pa[:, j0:j1],
                                    lhsT=ones_sb[:, :],
                                    rhs=mneg_sb[:, j0:j1],
                                    start=False,
                                    stop=True,
                                )
                            nc.scalar.activation(
                                out=expm,
                                in_=pa,
                                func=AF.Exp,
                                accum_out=S0[:, c : c + 1],
                            )
                            # p1 = expm*x with S1 = sum(p1) fused
                            p1 = prp.tile(
                                [128, TP], dt.bfloat16, tag="p1", name=f"p1_{s}_{c}"
                            )
                            nc.vector.scalar_tensor_tensor(
                                out=p1,
                                in0=expm,
                                scalar=1.0,
                                in1=xt,
                                op0=OP.mult,
                                op1=OP.mult,
                                accum_out=S1[:, c : c + 1],
                            )
                            if c in S2_ACT_CHUNKS:
                                # p2 on DVE (2x), reduce on ACT
                                p2 = prp.tile(
                                    [128, TP], dt.bfloat16, tag="p2",
                                    name=f"p2_{s}_{c}",
                                )
                                nc.vector.tensor_tensor(
                                    out=p2, in0=p1, in1=xt, op=OP.mult
                                )
                                junk = prp.tile(
                                    [128, TP], dt.bfloat16, tag="junk",
                                    name=f"junk_{s}_{c}",
                                )
                                nc.scalar.activation(
                                    out=junk,
                                    in_=p2,
                                    func=AF.Copy,
                                    accum_out=S2[:, c : c + 1],
                                )
                            else:
                                # fused p2+S2 on DVE
                                p2 = prp.tile(
                                    [128, TP], dt.bfloat16, tag="p2",
                                    name=f"p2_{s}_{c}",
                                )
                                nc.vector.scalar_tensor_tensor(
                                    out=p2,
                                    in0=p1,
                                    scalar=1.0,
                                    in1=xt,
                                    op0=OP.mult,
                                    op1=OP.mult,
                                    accum_out=S2[:, c : c + 1],
                                )

                tail_stage(SPC - 2)
                tail_stage(SPC - 1)

    nc.compile()
    nc.m = get_hw_module(nc.m)
    return nc


def _get_program(tp):
    key = ("nc", tp)
    if key not in _PROG_CACHE:
        _PROG_CACHE[key] = _build_program(tp=tp)
    return _PROG_CACHE[key]


def _prep_inputs(x, padding_mask, W_tdnn, b_tdnn, W_attn, b_attn):
    """Host-side prep: mask-compact x columns, cast/transpose, build
    per-core input maps."""
    x = np.asarray(x)
    padding_mask = np.asarray(padding_mask)
    keep = ~padding_mask  # (B, T) bool
    counts = keep.sum(axis=1)
    tp = int(counts.max())
    tp = max(256, -(-tp // 128) * 128)  # round up to 128 cols
    _STATE["tp"] = tp

    xc = np.zeros((B, C, tp), dtype=BF16)
    mneg = np.full((B, tp), -1e9, dtype=np.float32)
    for s in range(B):
        n = int(counts[s])
        xc[s, :, :n] = x[s][:, keep[s]].astype(BF16)
        mneg[s, :n] = 0.0
    mneg = mneg.astype(BF16)

    wt = np.ascontiguousarray(W_tdnn.T).astype(BF16)  # (C, BN)
    wa = np.ascontiguousarray(W_attn.T).astype(BF16)  # (BN, C)
    bt = np.ascontiguousarray(b_tdnn.astype(np.float32).reshape(BN, 1))
    in_maps = []
    for i in range(NCORES):
        sl = slice(i * SPC, (i + 1) * SPC)
        in_maps.append(
            {
                "x": np.ascontiguousarray(xc[sl]),
                "maskneg": np.ascontiguousarray(mneg[sl]),
                "wt": wt,
                "wa": wa,
                "bt": bt,
            }
        )
    return in_maps


def kernel(x, padding_mask, W_tdnn, b_tdnn, W_attn, b_attn):
    from concourse.bass_utils import run_bass_kernel_spmd

    in_maps = _prep_inputs(x, padding_mask, W_tdnn, b_tdnn, W_attn, b_attn)
    nc = _get_program(_STATE["tp"])
    res = run_bass_kernel_spmd(nc, in_maps, core_ids=list(range(NCORES)))
    out = np.concatenate([res.results[i]["out"] for i in range(NCORES)], axis=0)
    return out.astype(np.float32)


# revision 7
# speedup vs baseline: 1.6024x; 1.1634x over previous
"""AttentiveStatsPooling Trainium2 kernel.

Full-input contract: kernel(**inputs) takes the unsharded numpy inputs
  x            (32, 1536, 2048) f32
  padding_mask (32, 2048)       bool
  W_tdnn       (128, 1536)      f32
  b_tdnn       (128,)           f32
  W_attn       (1536, 128)      f32
  b_attn       (1536,)          f32
and returns the full (32, 3072) f32 output.

Sharding: data-parallel over batch. 8 cores x 4 samples each, weights
replicated. Math per sample:
  e    = tanh(W_tdnn @ x + b_tdnn)            (BN, T)
  a    = W_attn @ e  (+ b_attn: dropped - constant along T, cancels in
                      the softmax over T)      (C, T)
  a   += -1e9 * mask[t]                        (additive mask; exp -> 0)
  S0   = sum_t exp(a);  S1 = sum_t exp(a)*x;  S2 = sum_t exp(a)*x^2
  mean = S1/S0;  std = sqrt(clip(S2/S0 - mean^2, 1e-9))

Key optimizations over the naive schedule (HW-microbenchmarked):
  1. Host-side mask compaction: masked columns contribute exactly 0 to
     all three sums (exp(-1e9) == 0 and the sums are permutation-
     invariant), so only the unmasked columns of x are shipped and
     processed, zero-padded per-sample to a common width Tp (~56% of T
     for the ~50% random mask). Every per-element op and the x DMA
     shrink proportionally.
  2. Fused product+reduce: scalar_tensor_tensor computes
     out = in0*in1, accum_out = sum(out) in ONE 1x-rate DVE op
     (1.35us/[128,1152]), so S1 is free with the p1 product and S2 is
     one more such op. exp reads logits from PSUM and its ACT
     accumulator gives S0 free (1.45us).
  3. Engine balancing: a fraction of the p2 = p1*x products runs on the
     otherwise-idle Pool/GPSIMD engine (2.4us) with the S2 reduction on
     ACT (Copy+accum, 1.5us), tuned so ACT/DVE/Pool all stay busy.
  4. The S2 stage is software-pipelined DEPTH chunk-steps behind the
     exp/p1 stream so cross-engine chains never stall the hot engines.
For timing programs (reps=K), the body is unrolled UNROLL times inside
the hardware For_i loop: For_i inserts an all-engine barrier per
iteration, which drains the pipeline; unrolling amortizes that drain
while keeping the same per-rep work.
"""

import numpy as np
import ml_dtypes

B, C, T = 32, 1536, 2048
BN = 128
NCORES = 8
SPC = B // NCORES  # samples per core
CK = C // 128      # c chunks of 128 partitions

BF16 = ml_dtypes.bfloat16

# Default compacted width; _prep_inputs overrides from the actual mask.
_STATE = {"tp": 1152}

# Per-chunk S2 strategy (len CK): "dve" = fused product+reduce on DVE,
# "act" = p2 product on DVE (2x), reduce on ACT Copy+accum,
# "pool" = p2 product on the Pool/GPSIMD engine, reduce on ACT.
S2_MODE = ("dve", "pool", "dve", "pool", "dve", "pool",
           "dve", "pool", "dve", "pool", "dve", "dve")
# S2 reduce of chunk-step i is emitted after stage work of step i+DEPTH,
# so the reducing engine never stalls the exp/p1 pipeline.
DEPTH = 4
# Bodies per For_i iteration in timing (reps) programs.
UNROLL = 4

_PROG_CACHE = {}


def _build_program(reps=None, tp=None):
    """Build the per-core program. reps=None: straight-line body.
    reps=K: run the body K times on-device (For_i loop, unrolled)."""
    import concourse.bacc as bacc
    import concourse.tile as tile
    import concourse.mybir as mybir
    from concourse.bass_interp import get_hw_module

    dt = mybir.dt
    AF = mybir.ActivationFunctionType
    OP = mybir.AluOpType

    TP = int(tp if tp is not None else _STATE["tp"])
    # PSUM bank groups (free-dim tiles of <=512 fp32); PSUM tiles are
    # allocated at a whole-bank width so every pool slot stays
    # bank-aligned (matmul outputs must not straddle banks).
    banks = [(j, min(j + 512, TP)) for j in range(0, TP, 512)]
    PSW = -(-TP // 512) * 512

    nc = bacc.Bacc(
        "TRN2",
        target_bir_lowering=False,
        debug=False,
        num_devices=NCORES,
        num_swdge_queues=4,
    )
    x_d = nc.dram_tensor("x", [SPC, C, TP], dt.bfloat16, kind="ExternalInput")
    mn_d = nc.dram_tensor("maskneg", [SPC, TP], dt.bfloat16, kind="ExternalInput")
    wt_d = nc.dram_tensor("wt", [C, BN], dt.bfloat16, kind="ExternalInput")
    wa_d = nc.dram_tensor("wa", [BN, C], dt.bfloat16, kind="ExternalInput")
    bt_d = nc.dram_tensor("bt", [BN, 1], dt.float32, kind="ExternalInput")
    out_d = nc.dram_tensor("out", [SPC, 2 * C], dt.float32, kind="ExternalOutput")

    if reps is None:
        n_iter, unroll = None, 1
    else:
        unroll = next(k for k in (UNROLL, 2, 1) if reps % k == 0)
        n_iter = reps // unroll

    with tile.TileContext(nc) as tc:
        with (
            tc.tile_pool(name="const", bufs=1) as constp,
            tc.tile_pool(name="xin", bufs=4 * CK) as xp,
            tc.tile_pool(name="esb", bufs=3) as ep,
            tc.tile_pool(name="expm", bufs=4) as xpm,
            tc.tile_pool(name="prod", bufs=DEPTH + 3) as prp,
            tc.tile_pool(name="mneg", bufs=4) as mnp,
            tc.tile_pool(name="stats", bufs=2) as statsp,
            tc.tile_pool(name="tail", bufs=2) as tailp,
            tc.tile_pool(name="ps", bufs=2, space="PSUM") as psp,
        ):
            # ---- constants ------------------------------------------------
            wt_sb = constp.tile([128, CK, BN], dt.bfloat16, tag="wt")
            nc.sync.dma_start(
                out=wt_sb, in_=wt_d.ap().rearrange("(k p) o -> p k o", p=128)
            )
            wa_sb = constp.tile([128, C], dt.bfloat16, tag="wa")
            nc.sync.dma_start(out=wa_sb, in_=wa_d.ap())
            bt_sb = constp.tile([128, 1], dt.float32, tag="bt")
            nc.sync.dma_start(out=bt_sb, in_=bt_d.ap())
            ones_sb = constp.tile([1, 128], dt.bfloat16, tag="ones")
            nc.vector.memset(ones_sb, 1.0)

            def emit_body(u):
                stats = []
                for s in range(SPC):
                    S0 = statsp.tile(
                        [128, CK], dt.float32, tag=f"S0_{s}", name=f"S0_{u}_{s}"
                    )
                    S1 = statsp.tile(
                        [128, CK], dt.float32, tag=f"S1_{s}", name=f"S1_{u}_{s}"
                    )
                    S2 = statsp.tile(
                        [128, CK], dt.float32, tag=f"S2_{s}", name=f"S2_{u}_{s}"
                    )
                    stats.append((S0, S1, S2))

                def tail_stage(s):
                    # mean/std + output DMA for one finished sample
                    S0, S1, S2 = stats[s]
                    r0 = tailp.tile(
                        [128, CK], dt.float32, tag="r0", name=f"r0_{u}_{s}"
                    )
                    nc.vector.reciprocal(out=r0, in_=S0)
                    mean = tailp.tile(
                        [128, CK], dt.float32, tag="mean", name=f"mean_{u}_{s}"
                    )
                    nc.vector.tensor_tensor(out=mean, in0=S1, in1=r0, op=OP.mult)
                    ex2 = tailp.tile(
                        [128, CK], dt.float32, tag="ex2", name=f"ex2_{u}_{s}"
                    )
                    nc.vector.tensor_tensor(out=ex2, in0=S2, in1=r0, op=OP.mult)
                    m2 = tailp.tile(
                        [128, CK], dt.float32, tag="m2", name=f"m2_{u}_{s}"
                    )
                    nc.vector.tensor_tensor(out=m2, in0=mean, in1=mean, op=OP.mult)
                    var = tailp.tile(
                        [128, CK], dt.float32, tag="var", name=f"var_{u}_{s}"
                    )
                    nc.vector.tensor_tensor(out=var, in0=ex2, in1=m2, op=OP.subtract)
                    nc.vector.tensor_scalar(
                        out=var, in0=var, scalar1=1e-9, scalar2=None, op0=OP.max
                    )
                    std = tailp.tile(
                        [128, CK], dt.float32, tag="std", name=f"std_{u}_{s}"
                    )
                    nc.scalar.activation(out=std, in_=var, func=AF.Sqrt)
                    nc.sync.dma_start(
                        out=out_d.ap()[s, 0:C].rearrange("(ck p) -> p ck", p=128),
                        in_=mean,
                    )
                    nc.sync.dma_start(
                        out=out_d.ap()[s, C : 2 * C].rearrange(
                            "(ck p) -> p ck", p=128
                        ),
                        in_=std,
                    )

                # process samples in pairs; the two chunk streams interleave
                # so ACT/DVE always have an independent chunk to work on
                for s0 in range(0, SPC, 2):
                    pair = [s0, s0 + 1]
                    xts = {}
                    mnegs = {}
                    esbs = {}
                    for s in pair:
                        mneg_sb = mnp.tile(
                            [1, TP], dt.bfloat16, tag="mneg", name=f"mneg_{u}_{s}"
                        )
                        nc.sync.dma_start(out=mneg_sb, in_=mn_d.ap()[s : s + 1, :])
                        mnegs[s] = mneg_sb
                        for k in range(CK):
                            xt = xp.tile(
                                [128, TP], dt.bfloat16, tag="x",
                                name=f"x_{u}_{s}_{k}",
                            )
                            nc.sync.dma_start(
                                out=xt, in_=x_d.ap()[s, k * 128 : (k + 1) * 128, :]
                            )
                            xts[(s, k)] = xt

                    # mm1 + tanh for both samples of the pair
                    for s in pair:
                        pse_full = psp.tile(
                            [128, PSW], dt.float32, tag="ps", name=f"pse_{u}_{s}"
                        )
                        pse = pse_full[:, :TP]
                        for (j0, j1) in banks:
                            for k in range(CK):
                                nc.tensor.matmul(
                                    pse[:, j0:j1],
                                    lhsT=wt_sb[:, k, :],
                                    rhs=xts[(s, k)][:, j0:j1],
                                    start=(k == 0),
                                    stop=(k == CK - 1),
                                )
                        e_sb = ep.tile(
                            [128, TP], dt.bfloat16, tag="e", name=f"e_{u}_{s}"
                        )
                        nc.scalar.activation(
                            out=e_sb, in_=pse, func=AF.Tanh, bias=bt_sb, scale=1.0
                        )
                        esbs[s] = e_sb

                    # prior pair's tails overlap with this pair's chunk work
                    if s0 > 0:
                        tail_stage(s0 - 2)
                        tail_stage(s0 - 1)

                    def s2_stage(item):
                        s, c, p1, p2 = item
                        S2 = stats[s][2]
                        if S2_MODE[c] == "dve":
                            junk = prp.tile(
                                [128, TP], dt.bfloat16, tag="junk",
                                name=f"junk_{u}_{s}_{c}",
                            )
                            nc.vector.scalar_tensor_tensor(
                                out=junk,
                                in0=p1,
                                scalar=1.0,
                                in1=xts[(s, c)],
                                op0=OP.mult,
                                op1=OP.mult,
                                accum_out=S2[:, c : c + 1],
                            )
                        else:
                            junk = prp.tile(
                                [128, TP], dt.bfloat16, tag="junk",
                                name=f"junk_{u}_{s}_{c}",
                            )
                            nc.scalar.activation(
                                out=junk,
                                in_=p2,
                                func=AF.Copy,
                                accum_out=S2[:, c : c + 1],
                            )

                    pending = []
                    for c in range(CK):
                        for s in pair:
                            S0, S1, S2 = stats[s]
                            e_sb = esbs[s]
                            mneg_sb = mnegs[s]
                            xt = xts[(s, c)]
                            expm = xpm.tile(
                                [128, TP], dt.bfloat16, tag="expm",
                                name=f"expm_{u}_{s}_{c}",
                            )
                            pa_full = psp.tile(
                                [128, PSW], dt.float32, tag="ps",
                                name=f"pa_{u}_{s}_{c}",
                            )
                            pa = pa_full[:, :TP]
                            for (j0, j1) in banks:
                                nc.tensor.matmul(
                                    pa[:, j0:j1],
                                    lhsT=wa_sb[:, c * 128 : (c + 1) * 128],
                                    rhs=e_sb[:, j0:j1],
                                    start=True,
                                    stop=False,
                                )
                            for (j0, j1) in banks:
                                nc.tensor.matmul(
                                    pa[:, j0:j1],
                                    lhsT=ones_sb[:, :],
                                    rhs=mneg_sb[:, j0:j1],
                                    start=False,
                                    stop=True,
                                )
                            nc.scalar.activation(
                                out=expm,
                                in_=pa,
                                func=AF.Exp,
                                accum_out=S0[:, c : c + 1],
                            )
                            # p1 = expm*x with S1 = sum(p1) fused
                            p1 = prp.tile(
                                [128, TP], dt.bfloat16, tag="p1",
                                name=f"p1_{u}_{s}_{c}",
                            )
                            nc.vector.scalar_tensor_tensor(
                                out=p1,
                                in0=expm,
                                scalar=1.0,
                                in1=xt,
                                op0=OP.mult,
                                op1=OP.mult,
                                accum_out=S1[:, c : c + 1],
                            )
                            p2 = None
                            if S2_MODE[c] != "dve":
                                p2 = prp.tile(
                                    [128, TP], dt.bfloat16, tag="p2",
                                    name=f"p2_{u}_{s}_{c}",
                                )
                                eng = (
                                    nc.gpsimd if S2_MODE[c] == "pool" else nc.vector
                                )
                                eng.tensor_tensor(out=p2, in0=p1, in1=xt, op=OP.mult)
                            pending.append((s, c, p1, p2))
                            if len(pending) > DEPTH:
                                s2_stage(pending.pop(0))
                    for item in pending:
                        s2_stage(item)

                tail_stage(SPC - 2)
                tail_stage(SPC - 1)

            if n_iter is None:
                emit_body(0)
            else:
                with tc.For_i(0, n_iter, 1):
                    for u in range(unroll):
                        emit_body(u)

    nc.compile()
    nc.m = get_hw_module(nc.m)
    return nc


def _get_program(tp):
    key = ("nc", tp)
    if key not in _PROG_CACHE:
        _PROG_CACHE[key] = _build_program(tp=tp)
    return _PROG_CACHE[key]


def _prep_inputs(x, padding_mask, W_tdnn, b_tdnn, W_attn, b_attn):
    """Host-side prep: mask-compact x columns, cast/transpose, build
    per-core input maps."""
    x = np.asarray(x)
    padding_mask = np.asarray(padding_mask)
    keep = ~padding_mask  # (B, T) bool
    counts = keep.sum(axis=1)
    tp = int(counts.max())
    tp = max(256, -(-tp // 128) * 128)  # round up to 128 cols
    _STATE["tp"] = tp

    xc = np.zeros((B, C, tp), dtype=BF16)
    mneg = np.full((B, tp), -1e9, dtype=np.float32)
    for s in range(B):
        n = int(counts[s])
        xc[s, :, :n] = x[s][:, keep[s]].astype(BF16)
        mneg[s, :n] = 0.0
    mneg = mneg.astype(BF16)

    wt = np.ascontiguousarray(W_tdnn.T).astype(BF16)  # (C, BN)
    wa = np.ascontiguousarray(W_attn.T).astype(BF16)  # (BN, C)
    bt = np.ascontiguousarray(b_tdnn.astype(np.float32).reshape(BN, 1))
    in_maps = []
    for i in range(NCORES):
        sl = slice(i * SPC, (i + 1) * SPC)
        in_maps.append(
            {
                "x": np.ascontiguousarray(xc[sl]),
                "maskneg": np.ascontiguousarray(mneg[sl]),
                "wt": wt,
                "wa": wa,
                "bt": bt,
            }
        )
    return in_maps


def kernel(x, padding_mask, W_tdnn, b_tdnn, W_attn, b_attn):
    from concourse.bass_utils import run_bass_kernel_spmd

    in_maps = _prep_inputs(x, padding_mask, W_tdnn, b_tdnn, W_attn, b_attn)
    nc = _get_program(_STATE["tp"])
    res = run_bass_kernel_spmd(nc, in_maps, core_ids=list(range(NCORES)))
    out = np.concatenate([res.results[i]["out"] for i in range(NCORES)], axis=0)
    return out.astype(np.float32)


# revision 10
# speedup vs baseline: 2.5043x; 1.5628x over previous
"""AttentiveStatsPooling Trainium2 kernel.

Full-input contract: kernel(**inputs) takes the unsharded numpy inputs
  x            (32, 1536, 2048) f32
  padding_mask (32, 2048)       bool
  W_tdnn       (128, 1536)      f32
  b_tdnn       (128,)           f32
  W_attn       (1536, 128)      f32
  b_attn       (1536,)          f32
and returns the full (32, 3072) f32 output.

Sharding: data-parallel over batch. 8 cores x 4 samples each, weights
replicated. Math per sample:
  e    = tanh(W_tdnn @ x + b_tdnn)            (BN, T)
  a    = W_attn @ e  (+ b_attn: dropped - constant along T, cancels in
                      the softmax over T)      (C, T)
  a   += -1e9 * mask[t]                        (additive mask; exp -> 0)
  S0   = sum_t exp(a);  S1 = sum_t exp(a)*x;  S2 = sum_t exp(a)*x^2
  mean = S1/S0;  std = sqrt(clip(S2/S0 - mean^2, 1e-9))

Key optimizations over the naive schedule (HW-microbenchmarked):
  1. Host-side mask compaction: masked columns contribute exactly 0 to
     all three sums (exp(-1e9) == 0 and the sums are permutation-
     invariant), so only the unmasked columns of x are shipped and
     processed, zero-padded per-sample to a common width Tp (~56% of T
     for the ~50% random mask). Every per-element op and the x DMA
     shrink proportionally.
  2. Fused product+reduce: scalar_tensor_tensor computes
     out = in0*in1, accum_out = sum(out) in ONE 1x-rate DVE op
     (1.35us/[128,1152]), so S1 is free with the p1 product and S2 is
     one more such op. exp reads logits from PSUM and its ACT
     accumulator gives S0 free (1.45us).
  3. Engine balancing: a fraction of the p2 = p1*x products runs on the
     otherwise-idle Pool/GPSIMD engine (2.4us) with the S2 reduction on
     ACT (Copy+accum, 1.5us), tuned so ACT/DVE/Pool all stay busy.
  4. The S2 stage is software-pipelined DEPTH chunk-steps behind the
     exp/p1 stream so cross-engine chains never stall the hot engines.
For timing programs (reps=K), the body is unrolled UNROLL times inside
the hardware For_i loop: For_i inserts an all-engine barrier per
iteration, which drains the pipeline; unrolling amortizes that drain
while keeping the same per-rep work.
"""

import numpy as np
import ml_dtypes

B, C, T = 32, 1536, 2048
BN = 128
NCORES = 8
SPC = B // NCORES  # samples per core
CK = C // 128      # c chunks of 128 partitions

BF16 = ml_dtypes.bfloat16

# Default compacted width; _prep_inputs overrides from the actual mask.
_STATE = {"tp": 1152}

# Per-chunk S2 strategy (len CK): "dve" = fused product+reduce on DVE,
# "act" = p2 product on DVE (2x), reduce on ACT Copy+accum,
# "pool" = p2 product on the Pool/GPSIMD engine, reduce on ACT.
S2_MODE = ("act", "pool", "dve", "pool", "dve", "pool",
           "dve", "pool", "dve", "act", "dve", "dve")
# S2 reduce of chunk-step i is emitted after stage work of step i+DEPTH,
# so the reducing engine never stalls the exp/p1 pipeline.
DEPTH = 4
# Bodies per For_i iteration in timing (reps) programs.
UNROLL = 8

_PROG_CACHE = {}


def _build_program(reps=None, tp=None):
    """Build the per-core program. reps=None: straight-line body.
    reps=K: run the body K times on-device (For_i loop, unrolled)."""
    import concourse.bacc as bacc
    import concourse.tile as tile
    import concourse.mybir as mybir
    from concourse.bass_interp import get_hw_module

    dt = mybir.dt
    AF = mybir.ActivationFunctionType
    OP = mybir.AluOpType

    TP = int(tp if tp is not None else _STATE["tp"])
    # PSUM bank groups (free-dim tiles of <=512 fp32); PSUM tiles are
    # allocated at a whole-bank width so every pool slot stays
    # bank-aligned (matmul outputs must not straddle banks).
    banks = [(j, min(j + 512, TP)) for j in range(0, TP, 512)]
    PSW = -(-TP // 512) * 512

    nc = bacc.Bacc(
        "TRN2",
        target_bir_lowering=False,
        debug=False,
        num_devices=NCORES,
        num_swdge_queues=4,
    )
    x_d = nc.dram_tensor("x", [SPC, C, TP], dt.bfloat16, kind="ExternalInput")
    mn_d = nc.dram_tensor("maskneg", [SPC, TP], dt.bfloat16, kind="ExternalInput")
    wt_d = nc.dram_tensor("wt", [C, BN], dt.bfloat16, kind="ExternalInput")
    wa_d = nc.dram_tensor("wa", [BN, C], dt.bfloat16, kind="ExternalInput")
    bt_d = nc.dram_tensor("bt", [BN, 1], dt.float32, kind="ExternalInput")
    out_d = nc.dram_tensor("out", [SPC, 2 * C], dt.float32, kind="ExternalOutput")

    if reps is None:
        n_iter, unroll = None, 1
    else:
        unroll = next(k for k in (UNROLL, 8, 4, 2, 1) if reps % k == 0)
        n_iter = reps // unroll

    with tile.TileContext(nc) as tc:
        with (
            tc.tile_pool(name="const", bufs=1) as constp,
            tc.tile_pool(name="xin", bufs=4 * CK) as xp,
            tc.tile_pool(name="esb", bufs=4) as ep,
            tc.tile_pool(name="expm", bufs=4) as xpm,
            tc.tile_pool(name="prod", bufs=DEPTH + 3) as prp,
            tc.tile_pool(name="mneg", bufs=4) as mnp,
            tc.tile_pool(name="stats", bufs=2) as statsp,
            tc.tile_pool(name="tail", bufs=2) as tailp,
            tc.tile_pool(name="ps", bufs=2, space="PSUM") as psp,
            tc.tile_pool(name="pse", bufs=2, space="PSUM") as psep,
        ):
            # ---- constants ------------------------------------------------
            wt_sb = constp.tile([128, CK, BN], dt.bfloat16, tag="wt")
            nc.sync.dma_start(
                out=wt_sb, in_=wt_d.ap().rearrange("(k p) o -> p k o", p=128)
            )
            wa_sb = constp.tile([128, C], dt.bfloat16, tag="wa")
            nc.sync.dma_start(out=wa_sb, in_=wa_d.ap())
            bt_sb = constp.tile([128, 1], dt.float32, tag="bt")
            nc.sync.dma_start(out=bt_sb, in_=bt_d.ap())
            ones_sb = constp.tile([1, 128], dt.bfloat16, tag="ones")
            nc.vector.memset(ones_sb, 1.0)

            def emit_body(u):
                stats = []
                for s in range(SPC):
                    S0 = statsp.tile(
                        [128, CK], dt.float32, tag=f"S0_{s}", name=f"S0_{u}_{s}"
                    )
                    S1 = statsp.tile(
                        [128, CK], dt.float32, tag=f"S1_{s}", name=f"S1_{u}_{s}"
                    )
                    S2 = statsp.tile(
                        [128, CK], dt.float32, tag=f"S2_{s}", name=f"S2_{u}_{s}"
                    )
                    stats.append((S0, S1, S2))

                def tail_stage(s):
                    # mean/std + output DMA for one finished sample
                    S0, S1, S2 = stats[s]
                    r0 = tailp.tile(
                        [128, CK], dt.float32, tag="r0", name=f"r0_{u}_{s}"
                    )
                    nc.vector.reciprocal(out=r0, in_=S0)
                    mean = tailp.tile(
                        [128, CK], dt.float32, tag="mean", name=f"mean_{u}_{s}"
                    )
                    nc.vector.tensor_tensor(out=mean, in0=S1, in1=r0, op=OP.mult)
                    ex2 = tailp.tile(
                        [128, CK], dt.float32, tag="ex2", name=f"ex2_{u}_{s}"
                    )
                    nc.vector.tensor_tensor(out=ex2, in0=S2, in1=r0, op=OP.mult)
                    m2 = tailp.tile(
                        [128, CK], dt.float32, tag="m2", name=f"m2_{u}_{s}"
                    )
                    nc.vector.tensor_tensor(out=m2, in0=mean, in1=mean, op=OP.mult)
                    var = tailp.tile(
                        [128, CK], dt.float32, tag="var", name=f"var_{u}_{s}"
                    )
                    nc.vector.tensor_tensor(out=var, in0=ex2, in1=m2, op=OP.subtract)
                    nc.vector.tensor_scalar(
                        out=var, in0=var, scalar1=1e-9, scalar2=None, op0=OP.max
                    )
                    std = tailp.tile(
                        [128, CK], dt.float32, tag="std", name=f"std_{u}_{s}"
                    )
                    nc.scalar.activation(out=std, in_=var, func=AF.Sqrt)
                    nc.sync.dma_start(
                        out=out_d.ap()[s, 0:C].rearrange("(ck p) -> p ck", p=128),
                        in_=mean,
                    )
                    nc.sync.dma_start(
                        out=out_d.ap()[s, C : 2 * C].rearrange(
                            "(ck p) -> p ck", p=128
                        ),
                        in_=std,
                    )

                # process samples in pairs; the two chunk streams interleave
                # so ACT/DVE always have an independent chunk to work on.
                # The NEXT pair's x DMA and mm1+tanh run during the current
                # pair's chunk stream (mm1 uses its own 1-bank PSUM piece
                # pool so it never contends with the mm2 pa tiles).
                xts = {}
                mnegs = {}
                esbs = {}

                def emit_dma(pair):
                    for s in pair:
                        mneg_sb = mnp.tile(
                            [1, TP], dt.bfloat16, tag="mneg", name=f"mneg_{u}_{s}"
                        )
                        nc.sync.dma_start(out=mneg_sb, in_=mn_d.ap()[s : s + 1, :])
                        mnegs[s] = mneg_sb
                        for k in range(CK):
                            xt = xp.tile(
                                [128, TP], dt.bfloat16, tag="x",
                                name=f"x_{u}_{s}_{k}",
                            )
                            nc.sync.dma_start(
                                out=xt, in_=x_d.ap()[s, k * 128 : (k + 1) * 128, :]
                            )
                            xts[(s, k)] = xt

                def emit_mm1(s):
                    # piecewise mm1 + tanh (one PSUM bank per piece)
                    e_sb = ep.tile(
                        [128, TP], dt.bfloat16, tag="e", name=f"e_{u}_{s}"
                    )
                    for pj, (j0, j1) in enumerate(banks):
                        w = j1 - j0
                        pse = psep.tile(
                            [128, 512], dt.float32, tag="pse",
                            name=f"pse_{u}_{s}_{pj}",
                        )
                        for k in range(CK):
                            nc.tensor.matmul(
                                pse[:, :w],
                                lhsT=wt_sb[:, k, :],
                                rhs=xts[(s, k)][:, j0:j1],
                                start=(k == 0),
                                stop=(k == CK - 1),
                            )
                        nc.scalar.activation(
                            out=e_sb[:, j0:j1],
                            in_=pse[:, :w],
                            func=AF.Tanh,
                            bias=bt_sb,
                            scale=1.0,
                        )
                    esbs[s] = e_sb

                def s2_stage(item):
                    s, c, p1, p2 = item
                    S2 = stats[s][2]
                    junk = prp.tile(
                        [128, TP], dt.bfloat16, tag="junk",
                        name=f"junk_{u}_{s}_{c}",
                    )
                    if S2_MODE[c] == "dve":
                        nc.vector.scalar_tensor_tensor(
                            out=junk,
                            in0=p1,
                            scalar=1.0,
                            in1=xts[(s, c)],
                            op0=OP.mult,
                            op1=OP.mult,
                            accum_out=S2[:, c : c + 1],
                        )
                    else:
                        nc.scalar.activation(
                            out=junk,
                            in_=p2,
                            func=AF.Copy,
                            accum_out=S2[:, c : c + 1],
                        )

                pairs = [[s0, s0 + 1] for s0 in range(0, SPC, 2)]
                emit_dma(pairs[0])
                for s in pairs[0]:
                    emit_mm1(s)

                pending = []
                tail_at = -(-DEPTH // 2)  # prior pair's S2s drained by then
                for pi, pair in enumerate(pairs):
                    for c in range(CK):
                        # prior pair's tails overlap with this pair's work;
                        # by chunk `tail_at` the pending queue has flushed
                        # every S2 belonging to the previous pair
                        if pi > 0 and c == tail_at:
                            tail_stage(pair[0] - 2)
                            tail_stage(pair[0] - 1)
                        for s in pair:
                            S0, S1, S2 = stats[s]
                            e_sb = esbs[s]
                            mneg_sb = mnegs[s]
                            xt = xts[(s, c)]
                            expm = xpm.tile(
                                [128, TP], dt.bfloat16, tag="expm",
                                name=f"expm_{u}_{s}_{c}",
                            )
                            pa_full = psp.tile(
                                [128, PSW], dt.float32, tag="ps",
                                name=f"pa_{u}_{s}_{c}",
                            )
                            pa = pa_full[:, :TP]
                            for (j0, j1) in banks:
                                nc.tensor.matmul(
                                    pa[:, j0:j1],
                                    lhsT=wa_sb[:, c * 128 : (c + 1) * 128],
                                    rhs=e_sb[:, j0:j1],
                                    start=True,
                                    stop=False,
                                )
                            for (j0, j1) in banks:
                                nc.tensor.matmul(
                                    pa[:, j0:j1],
                                    lhsT=ones_sb[:, :],
                                    rhs=mneg_sb[:, j0:j1],
                                    start=False,
                                    stop=True,
                                )
                            nc.scalar.activation(
                                out=expm,
                                in_=pa,
                                func=AF.Exp,
                                accum_out=S0[:, c : c + 1],
                            )
                            # p1 = expm*x with S1 = sum(p1) fused
                            p1 = prp.tile(
                                [128, TP], dt.bfloat16, tag="p1",
                                name=f"p1_{u}_{s}_{c}",
                            )
                            nc.vector.scalar_tensor_tensor(
                                out=p1,
                                in0=expm,
                                scalar=1.0,
                                in1=xt,
                                op0=OP.mult,
                                op1=OP.mult,
                                accum_out=S1[:, c : c + 1],
                            )
                            p2 = None
                            if S2_MODE[c] != "dve":
                                p2 = prp.tile(
                                    [128, TP], dt.bfloat16, tag="p2",
                                    name=f"p2_{u}_{s}_{c}",
                                )
                                eng = (
                                    nc.gpsimd if S2_MODE[c] == "pool" else nc.vector
                                )
                                eng.tensor_tensor(out=p2, in0=p1, in1=xt, op=OP.mult)
                            pending.append((s, c, p1, p2))
                            if len(pending) > DEPTH:
                                s2_stage(pending.pop(0))
                        # prefetch / pre-compute for the next pair
                        if pi + 1 < len(pairs):
                            if c == 1:
                                emit_dma(pairs[pi + 1])
                            elif c == 8:
                                emit_mm1(pairs[pi + 1][0])
                            elif c == 10:
                                emit_mm1(pairs[pi + 1][1])
                for item in pending:
                    s2_stage(item)

                tail_stage(SPC - 2)
                tail_stage(SPC - 1)

            if n_iter is None:
                emit_body(0)
            else:
                with tc.For_i(0, n_iter, 1):
                    for u in range(unroll):
                        emit_body(u)

    nc.compile()
    nc.m = get_hw_module(nc.m)
    return nc


def _get_program(tp):
    key = ("nc", tp)
    if key not in _PROG_CACHE:
        _PROG_CACHE[key] = _build_program(tp=tp)
    return _PROG_CACHE[key]


def _prep_inputs(x, padding_mask, W_tdnn, b_tdnn, W_attn, b_attn):
    """Host-side prep: mask-compact x columns, cast/transpose, build
    per-core input maps."""
    x = np.asarray(x)
    padding_mask = np.asarray(padding_mask)
    keep = ~padding_mask  # (B, T) bool
    counts = keep.sum(axis=1)
    tp = int(counts.max())
    tp = max(256, -(-tp // 128) * 128)  # round up to 128 cols
    _STATE["tp"] = tp

    xc = np.zeros((B, C, tp), dtype=BF16)
    mneg = np.full((B, tp), -1e9, dtype=np.float32)
    for s in range(B):
        n = int(counts[s])
        xc[s, :, :n] = x[s][:, keep[s]].astype(BF16)
        mneg[s, :n] = 0.0
    mneg = mneg.astype(BF16)

    wt = np.ascontiguousarray(W_tdnn.T).astype(BF16)  # (C, BN)
    wa = np.ascontiguousarray(W_attn.T).astype(BF16)  # (BN, C)
    bt = np.ascontiguousarray(b_tdnn.astype(np.float32).reshape(BN, 1))
    in_maps = []
    for i in range(NCORES):
        sl = slice(i * SPC, (i + 1) * SPC)
        in_maps.append(
            {
                "x": np.ascontiguousarray(xc[sl]),
                "maskneg": np.ascontiguousarray(mneg[sl]),
                "wt": wt,
                "wa": wa,
                "bt": bt,
            }
        )
    return in_maps


def kernel(x, padding_mask, W_tdnn, b_tdnn, W_attn, b_attn):
    from concourse.bass_utils import run_bass_kernel_spmd

    in_maps = _prep_inputs(x, padding_mask, W_tdnn, b_tdnn, W_attn, b_attn)
    nc = _get_program(_STATE["tp"])
    res = run_bass_kernel_spmd(nc, in_maps, core_ids=list(range(NCORES)))
    out = np.concatenate([res.results[i]["out"] for i in range(NCORES)], axis=0)
    return out.astype(np.float32)


# revision 12
# speedup vs baseline: 2.5213x; 1.0068x over previous
"""AttentiveStatsPooling Trainium2 kernel.

Full-input contract: kernel(**inputs) takes the unsharded numpy inputs
  x            (32, 1536, 2048) f32
  padding_mask (32, 2048)       bool
  W_tdnn       (128, 1536)      f32
  b_tdnn       (128,)           f32
  W_attn       (1536, 128)      f32
  b_attn       (1536,)          f32
and returns the full (32, 3072) f32 output.

Sharding: data-parallel over batch. 8 cores x 4 samples each, weights
replicated. Math per sample:
  e    = tanh(W_tdnn @ x + b_tdnn)            (BN, T)
  a    = W_attn @ e  (+ b_attn: dropped - constant along T, cancels in
                      the softmax over T)      (C, T)
  a   += -1e9 * mask[t]                        (additive mask; exp -> 0)
  S0   = sum_t exp(a);  S1 = sum_t exp(a)*x;  S2 = sum_t exp(a)*x^2
  mean = S1/S0;  std = sqrt(clip(S2/S0 - mean^2, 1e-9))

Key optimizations over the naive schedule (HW-microbenchmarked):
  1. Host-side mask compaction: masked columns contribute exactly 0 to
     all three sums (exp(-1e9) == 0 and the sums are permutation-
     invariant), so only the unmasked columns of x are shipped and
     processed, zero-padded per-sample to a common width Tp (~56% of T
     for the ~50% random mask). Every per-element op and the x DMA
     shrink proportionally.
  2. Fused product+reduce: scalar_tensor_tensor computes
     out = in0*in1, accum_out = sum(out) in ONE 1x-rate DVE op
     (1.35us/[128,1152]), so S1 is free with the p1 product and S2 is
     one more such op. exp reads logits from PSUM and its ACT
     accumulator gives S0 free (1.45us).
  3. Engine balancing: a fraction of the p2 = p1*x products runs on the
     otherwise-idle Pool/GPSIMD engine (2.4us) with the S2 reduction on
     ACT (Copy+accum, 1.5us), tuned so ACT/DVE/Pool all stay busy.
  4. The S2 stage is software-pipelined DEPTH chunk-steps behind the
     exp/p1 stream so cross-engine chains never stall the hot engines.
  5. The next pair's x DMA and its mm1+tanh (run piecewise through a
     dedicated 1-bank PSUM pool, so it never contends with the mm2
     accumulators) are issued during the current pair's chunk stream,
     removing the pair-boundary stalls on ACT/DVE.
For timing programs (reps=K), the body is unrolled UNROLL times inside
the hardware For_i loop: For_i inserts an all-engine barrier per
iteration, which drains the pipeline (~76us/iteration); unrolling
amortizes that drain while keeping the same per-rep work (K bodies
execute per loop trip, reps total).

Measured (8-core SPMD, per-body): 148us vs 343us for the previous
tensor_tensor + tensor_reduce schedule on the same estimator (2.3x);
rel l2 error vs the f32 reference: 2.0e-4.
"""

import numpy as np
import ml_dtypes

B, C, T = 32, 1536, 2048
BN = 128
NCORES = 8
SPC = B // NCORES  # samples per core
CK = C // 128      # c chunks of 128 partitions

BF16 = ml_dtypes.bfloat16

# Default compacted width; _prep_inputs overrides from the actual mask.
_STATE = {"tp": 1152}

# Per-chunk S2 strategy (len CK): "dve" = fused product+reduce on DVE,
# "act" = p2 product on DVE (2x), reduce on ACT Copy+accum,
# "pool" = p2 product on the Pool/GPSIMD engine, reduce on ACT.
S2_MODE = ("act", "pool", "dve", "pool", "dve", "pool",
           "dve", "pool", "dve", "act", "dve", "dve")
# S2 reduce of chunk-step i is emitted after stage work of step i+DEPTH,
# so the reducing engine never stalls the exp/p1 pipeline.
DEPTH = 4
# Bodies per For_i iteration in timing (reps) programs.
UNROLL = 8
# Chunk-step positions (within a pair's 12 chunk steps) at which the next
# pair's x DMA and its two mm1+tanh stages are emitted.
DMA_C = 1
MM1_C = (8, 10)

_PROG_CACHE = {}


def _build_program(reps=None, tp=None):
    """Build the per-core program. reps=None: straight-line body.
    reps=K: run the body K times on-device (For_i loop, unrolled)."""
    import concourse.bacc as bacc
    import concourse.tile as tile
    import concourse.mybir as mybir
    from concourse.bass_interp import get_hw_module

    dt = mybir.dt
    AF = mybir.ActivationFunctionType
    OP = mybir.AluOpType

    TP = int(tp if tp is not None else _STATE["tp"])
    # PSUM bank groups (free-dim tiles of <=512 fp32); PSUM tiles are
    # allocated at a whole-bank width so every pool slot stays
    # bank-aligned (matmul outputs must not straddle banks).
    banks = [(j, min(j + 512, TP)) for j in range(0, TP, 512)]
    PSW = -(-TP // 512) * 512

    nc = bacc.Bacc(
        "TRN2",
        target_bir_lowering=False,
        debug=False,
        num_devices=NCORES,
        num_swdge_queues=4,
    )
    x_d = nc.dram_tensor("x", [SPC, C, TP], dt.bfloat16, kind="ExternalInput")
    mn_d = nc.dram_tensor("maskneg", [SPC, TP], dt.bfloat16, kind="ExternalInput")
    wt_d = nc.dram_tensor("wt", [C, BN], dt.bfloat16, kind="ExternalInput")
    wa_d = nc.dram_tensor("wa", [BN, C], dt.bfloat16, kind="ExternalInput")
    bt_d = nc.dram_tensor("bt", [BN, 1], dt.float32, kind="ExternalInput")
    out_d = nc.dram_tensor("out", [SPC, 2 * C], dt.float32, kind="ExternalOutput")

    if reps is None:
        n_iter, unroll = None, 1
    else:
        unroll = next(k for k in (UNROLL, 8, 4, 2, 1) if reps % k == 0)
        n_iter = reps // unroll

    with tile.TileContext(nc) as tc:
        with (
            tc.tile_pool(name="const", bufs=1) as constp,
            tc.tile_pool(name="xin", bufs=4 * CK) as xp,
            tc.tile_pool(name="esb", bufs=4) as ep,
            tc.tile_pool(name="expm", bufs=4) as xpm,
            tc.tile_pool(name="prod", bufs=DEPTH + 3) as prp,
            tc.tile_pool(name="mneg", bufs=4) as mnp,
            tc.tile_pool(name="stats", bufs=2) as statsp,
            tc.tile_pool(name="tail", bufs=2) as tailp,
            tc.tile_pool(name="ps", bufs=2, space="PSUM") as psp,
            tc.tile_pool(name="pse", bufs=2, space="PSUM") as psep,
        ):
            # ---- constants ------------------------------------------------
            wt_sb = constp.tile([128, CK, BN], dt.bfloat16, tag="wt")
            nc.sync.dma_start(
                out=wt_sb, in_=wt_d.ap().rearrange("(k p) o -> p k o", p=128)
            )
            wa_sb = constp.tile([128, C], dt.bfloat16, tag="wa")
            nc.sync.dma_start(out=wa_sb, in_=wa_d.ap())
            bt_sb = constp.tile([128, 1], dt.float32, tag="bt")
            nc.sync.dma_start(out=bt_sb, in_=bt_d.ap())
            ones_sb = constp.tile([1, 128], dt.bfloat16, tag="ones")
            nc.vector.memset(ones_sb, 1.0)

            def emit_body(u):
                stats = []
                for s in range(SPC):
                    S0 = statsp.tile(
                        [128, CK], dt.float32, tag=f"S0_{s}", name=f"S0_{u}_{s}"
                    )
                    S1 = statsp.tile(
                        [128, CK], dt.float32, tag=f"S1_{s}", name=f"S1_{u}_{s}"
                    )
                    S2 = statsp.tile(
                        [128, CK], dt.float32, tag=f"S2_{s}", name=f"S2_{u}_{s}"
                    )
                    stats.append((S0, S1, S2))

                def tail_stage(s):
                    # mean/std + output DMA for one finished sample
                    S0, S1, S2 = stats[s]
                    r0 = tailp.tile(
                        [128, CK], dt.float32, tag="r0", name=f"r0_{u}_{s}"
                    )
                    nc.vector.reciprocal(out=r0, in_=S0)
                    mean = tailp.tile(
                        [128, CK], dt.float32, tag="mean", name=f"mean_{u}_{s}"
                    )
                    nc.vector.tensor_tensor(out=mean, in0=S1, in1=r0, op=OP.mult)
                    ex2 = tailp.tile(
                        [128, CK], dt.float32, tag="ex2", name=f"ex2_{u}_{s}"
                    )
                    nc.vector.tensor_tensor(out=ex2, in0=S2, in1=r0, op=OP.mult)
                    m2 = tailp.tile(
                        [128, CK], dt.float32, tag="m2", name=f"m2_{u}_{s}"
                    )
                    nc.vector.tensor_tensor(out=m2, in0=mean, in1=mean, op=OP.mult)
                    var = tailp.tile(
                        [128, CK], dt.float32, tag="var", name=f"var_{u}_{s}"
                    )
                    nc.vector.tensor_tensor(out=var, in0=ex2, in1=m2, op=OP.subtract)
                    nc.vector.tensor_scalar(
                        out=var, in0=var, scalar1=1e-9, scalar2=None, op0=OP.max
                    )
                    std = tailp.tile(
                        [128, CK], dt.float32, tag="std", name=f"std_{u}_{s}"
                    )
                    nc.scalar.activation(out=std, in_=var, func=AF.Sqrt)
                    nc.sync.dma_start(
                        out=out_d.ap()[s, 0:C].rearrange("(ck p) -> p ck", p=128),
                        in_=mean,
                    )
                    nc.sync.dma_start(
                        out=out_d.ap()[s, C : 2 * C].rearrange(
                            "(ck p) -> p ck", p=128
                        ),
                        in_=std,
                    )

                # process samples in pairs; the two chunk streams interleave
                # so ACT/DVE always have an independent chunk to work on.
                # The NEXT pair's x DMA and mm1+tanh run during the current
                # pair's chunk stream (mm1 uses its own 1-bank PSUM piece
                # pool so it never contends with the mm2 pa tiles).
                xts = {}
                mnegs = {}
                esbs = {}

                def emit_dma(pair):
                    for s in pair:
                        mneg_sb = mnp.tile(
                            [1, TP], dt.bfloat16, tag="mneg", name=f"mneg_{u}_{s}"
                        )
                        nc.sync.dma_start(out=mneg_sb, in_=mn_d.ap()[s : s + 1, :])
                        mnegs[s] = mneg_sb
                        for k in range(CK):
                            xt = xp.tile(
                                [128, TP], dt.bfloat16, tag="x",
                                name=f"x_{u}_{s}_{k}",
                            )
                            nc.sync.dma_start(
                                out=xt, in_=x_d.ap()[s, k * 128 : (k + 1) * 128, :]
                            )
                            xts[(s, k)] = xt

                def emit_mm1(s):
                    # piecewise mm1 + tanh (one PSUM bank per piece)
                    e_sb = ep.tile(
                        [128, TP], dt.bfloat16, tag="e", name=f"e_{u}_{s}"
                    )
                    for pj, (j0, j1) in enumerate(banks):
                        w = j1 - j0
                        pse = psep.tile(
                            [128, 512], dt.float32, tag="pse",
                            name=f"pse_{u}_{s}_{pj}",
                        )
                        for k in range(CK):
                            nc.tensor.matmul(
                                pse[:, :w],
                                lhsT=wt_sb[:, k, :],
                                rhs=xts[(s, k)][:, j0:j1],
                                start=(k == 0),
                                stop=(k == CK - 1),
                            )
                        nc.scalar.activation(
                            out=e_sb[:, j0:j1],
                            in_=pse[:, :w],
                            func=AF.Tanh,
                            bias=bt_sb,
                            scale=1.0,
                        )
                    esbs[s] = e_sb

                def s2_stage(item):
                    s, c, p1, p2 = item
                    S2 = stats[s][2]
                    junk = prp.tile(
                        [128, TP], dt.bfloat16, tag="junk",
                        name=f"junk_{u}_{s}_{c}",
                    )
                    if S2_MODE[c] == "dve":
                        nc.vector.scalar_tensor_tensor(
                            out=junk,
                            in0=p1,
                            scalar=1.0,
                            in1=xts[(s, c)],
                            op0=OP.mult,
                            op1=OP.mult,
                            accum_out=S2[:, c : c + 1],
                        )
                    else:
                        nc.scalar.activation(
                            out=junk,
                            in_=p2,
                            func=AF.Copy,
                            accum_out=S2[:, c : c + 1],
                        )

                pairs = [[s0, s0 + 1] for s0 in range(0, SPC, 2)]
                emit_dma(pairs[0])
                for s in pairs[0]:
                    emit_mm1(s)

                pending = []
                tail_at = -(-DEPTH // 2)  # prior pair's S2s drained by then
                for pi, pair in enumerate(pairs):
                    for c in range(CK):
                        # prior pair's tails overlap with this pair's work;
                        # by chunk `tail_at` the pending queue has flushed
                        # every S2 belonging to the previous pair
                        if pi > 0 and c == tail_at:
                            tail_stage(pair[0] - 2)
                            tail_stage(pair[0] - 1)
                        for s in pair:
                            S0, S1, S2 = stats[s]
                            e_sb = esbs[s]
                            mneg_sb = mnegs[s]
                            xt = xts[(s, c)]
                            expm = xpm.tile(
                                [128, TP], dt.bfloat16, tag="expm",
                                name=f"expm_{u}_{s}_{c}",
                            )
                            pa_full = psp.tile(
                                [128, PSW], dt.float32, tag="ps",
                                name=f"pa_{u}_{s}_{c}",
                            )
                            pa = pa_full[:, :TP]
                            for (j0, j1) in banks:
                                nc.tensor.matmul(
                                    pa[:, j0:j1],
                                    lhsT=wa_sb[:, c * 128 : (c + 1) * 128],
                                    rhs=e_sb[:, j0:j1],
                                    start=True,
                                    stop=False,
                                )
                            for (j0, j1) in banks:
                                nc.tensor.matmul(
                                    pa[:, j0:j1],
                                    lhsT=ones_sb[:, :],
                                    rhs=mneg_sb[:, j0:j1],
                                    start=False,
                                    stop=True,
                                )
                            nc.scalar.activation(
                                out=expm,
                                in_=pa,
                                func=AF.Exp,
                                accum_out=S0[:, c : c + 1],
                            )
                            # p1 = expm*x with S1 = sum(p1) fused
                            p1 = prp.tile(
                                [128, TP], dt.bfloat16, tag="p1",
                                name=f"p1_{u}_{s}_{c}",
                            )
                            nc.vector.scalar_tensor_tensor(
                                out=p1,
                                in0=expm,
                                scalar=1.0,
                                in1=xt,
                                op0=OP.mult,
                                op1=OP.mult,
                                accum_out=S1[:, c : c + 1],
                            )
                            p2 = None
                            if S2_MODE[c] != "dve":
                                p2 = prp.tile(
                                    [128, TP], dt.bfloat16, tag="p2",
                                    name=f"p2_{u}_{s}_{c}",
                                )
                                eng = (
                                    nc.gpsimd if S2_MODE[c] == "pool" else nc.vector
                                )
                                eng.tensor_tensor(out=p2, in0=p1, in1=xt, op=OP.mult)
                            pending.append((s, c, p1, p2))
                            if len(pending) > DEPTH:
                                s2_stage(pending.pop(0))
                        # prefetch / pre-compute for the next pair
                        if pi + 1 < len(pairs):
                            if c == DMA_C:
                                emit_dma(pairs[pi + 1])
                            elif c == MM1_C[0]:
                                emit_mm1(pairs[pi + 1][0])
                            elif c == MM1_C[1]:
                                emit_mm1(pairs[pi + 1][1])
                for item in pending:
                    s2_stage(item)

                tail_stage(SPC - 2)
                tail_stage(SPC - 1)

            if n_iter is None:
                emit_body(0)
            else:
                with tc.For_i(0, n_iter, 1):
                    for u in range(unroll):
                        emit_body(u)

    nc.compile()
    nc.m = get_hw_module(nc.m)
    return nc


def _get_program(tp):
    key = ("nc", tp)
    if key not in _PROG_CACHE:
        _PROG_CACHE[key] = _build_program(tp=tp)
    return _PROG_CACHE[key]


def _prep_inputs(x, padding_mask, W_tdnn, b_tdnn, W_attn, b_attn):
    """Host-side prep: mask-compact x columns, cast/transpose, build
    per-core input maps."""
    x = np.asarray(x)
    padding_mask = np.asarray(padding_mask)
    keep = ~padding_mask  # (B, T) bool
    counts = keep.sum(axis=1)
    tp = int(counts.max())
    tp = max(256, -(-tp // 128) * 128)  # round up to 128 cols
    _STATE["tp"] = tp

    xc = np.zeros((B, C, tp), dtype=BF16)
    mneg = np.full((B, tp), -1e9, dtype=np.float32)
    for s in range(B):
        n = int(counts[s])
        xc[s, :, :n] = x[s][:, keep[s]].astype(BF16)
        mneg[s, :n] = 0.0
    mneg = mneg.astype(BF16)

    wt = np.ascontiguousarray(W_tdnn.T).astype(BF16)  # (C, BN)
    wa = np.ascontiguousarray(W_attn.T).astype(BF16)  # (BN, C)
    bt = np.ascontiguousarray(b_tdnn.astype(np.float32).reshape(BN, 1))
    in_maps = []
    for i in range(NCORES):
        sl = slice(i * SPC, (i + 1) * SPC)
        in_maps.append(
            {
                "x": np.ascontiguousarray(xc[sl]),
                "maskneg": np.ascontiguousarray(mneg[sl]),
                "wt": wt,
                "wa": wa,
                "bt": bt,
            }
        )
    return in_maps


def kernel(x, padding_mask, W_tdnn, b_tdnn, W_attn, b_attn):
    from concourse.bass_utils import run_bass_kernel_spmd

    in_maps = _prep_inputs(x, padding_mask, W_tdnn, b_tdnn, W_attn, b_attn)
    nc = _get_program(_STATE["tp"])
    res = run_bass_kernel_spmd(nc, in_maps, core_ids=list(range(NCORES)))
    out = np.concatenate([res.results[i]["out"] for i in range(NCORES)], axis=0)
    return out.astype(np.float32)


# revision 14
# speedup vs baseline: 2.5500x; 1.0114x over previous
"""AttentiveStatsPooling Trainium2 kernel.

Full-input contract: kernel(**inputs) takes the unsharded numpy inputs
  x            (32, 1536, 2048) f32
  padding_mask (32, 2048)       bool
  W_tdnn       (128, 1536)      f32
  b_tdnn       (128,)           f32
  W_attn       (1536, 128)      f32
  b_attn       (1536,)          f32
and returns the full (32, 3072) f32 output.

Sharding: data-parallel over batch. 8 cores x 4 samples each, weights
replicated. Math per sample:
  e    = tanh(W_tdnn @ x + b_tdnn)            (BN, T)
  a    = W_attn @ e  (+ b_attn: dropped - constant along T, cancels in
                      the softmax over T)      (C, T)
  a   += -1e9 * mask[t]                        (additive mask; exp -> 0)
  S0   = sum_t exp(a);  S1 = sum_t exp(a)*x;  S2 = sum_t exp(a)*x^2
  mean = S1/S0;  std = sqrt(clip(S2/S0 - mean^2, 1e-9))

Key optimizations over the naive schedule (HW-microbenchmarked):
  1. Host-side mask compaction: masked columns contribute exactly 0 to
     all three sums (exp(-1e9) == 0 and the sums are permutation-
     invariant), so only the unmasked columns of x are shipped and
     processed, zero-padded per-sample to a common width Tp (~56% of T
     for the ~50% random mask). Every per-element op and the x DMA
     shrink proportionally.
  2. Fused product+reduce: scalar_tensor_tensor computes
     out = in0*in1, accum_out = sum(out) in ONE 1x-rate DVE op
     (1.35us/[128,1152]), so S1 is free with the p1 product and S2 is
     one more such op. exp reads logits from PSUM and its ACT
     accumulator gives S0 free (1.45us).
  3. Engine balancing: a fraction of the p2 = p1*x products runs on the
     otherwise-idle Pool/GPSIMD engine (2.4us) with the S2 reduction on
     ACT (Copy+accum, 1.5us), tuned so ACT/DVE/Pool all stay busy.
  4. The S2 stage is software-pipelined DEPTH chunk-steps behind the
     exp/p1 stream so cross-engine chains never stall the hot engines.
  5. The next pair's x DMA and its mm1+tanh (run piecewise through a
     dedicated 1-bank PSUM pool, so it never contends with the mm2
     accumulators) are issued during the current pair's chunk stream,
     removing the pair-boundary stalls on ACT/DVE.
For timing programs (reps=K), the body is unrolled UNROLL times inside
the hardware For_i loop: For_i inserts an all-engine barrier per
iteration, which drains the pipeline (~76us/iteration); unrolling
amortizes that drain while keeping the same per-rep work (K bodies
execute per loop trip, reps total).

Measured (8-core SPMD, per-body): 148us vs 343us for the previous
tensor_tensor + tensor_reduce schedule on the same estimator (2.3x);
rel l2 error vs the f32 reference: 2.0e-4.
"""

import numpy as np
import ml_dtypes

B, C, T = 32, 1536, 2048
BN = 128
NCORES = 8
SPC = B // NCORES  # samples per core
CK = C // 128      # c chunks of 128 partitions

BF16 = ml_dtypes.bfloat16

# Default compacted width; _prep_inputs overrides from the actual mask.
_STATE = {"tp": 1152}

# Per-chunk S2 strategy (len CK): "dve" = fused product+reduce on DVE,
# "act" = p2 product on DVE (2x), reduce on ACT Copy+accum,
# "pool" = p2 product on the Pool/GPSIMD engine, reduce on ACT.
S2_MODE = ("dve", "pool", "dve", "act", "pool", "dve",
           "pool", "dve", "act", "dve", "pool", "dve")
# S2 reduce of chunk-step i is emitted after stage work of step i+DEPTH,
# so the reducing engine never stalls the exp/p1 pipeline.
DEPTH = 4
# Bodies per For_i iteration in timing (reps) programs.
UNROLL = 8
# Chunk-step positions (within a pair's 12 chunk steps) at which the next
# pair's x DMA and its two mm1+tanh stages are emitted.
DMA_C = 1
MM1_C = (8, 10)
# SWDGE queue count for the build.
NQUEUES = 4

_PROG_CACHE = {}


def _build_program(reps=None, tp=None):
    """Build the per-core program. reps=None: straight-line body.
    reps=K: run the body K times on-device (For_i loop, unrolled)."""
    import concourse.bacc as bacc
    import concourse.tile as tile
    import concourse.mybir as mybir
    from concourse.bass_interp import get_hw_module

    dt = mybir.dt
    AF = mybir.ActivationFunctionType
    OP = mybir.AluOpType

    TP = int(tp if tp is not None else _STATE["tp"])
    # PSUM bank groups (free-dim tiles of <=512 fp32); PSUM tiles are
    # allocated at a whole-bank width so every pool slot stays
    # bank-aligned (matmul outputs must not straddle banks).
    banks = [(j, min(j + 512, TP)) for j in range(0, TP, 512)]
    PSW = -(-TP // 512) * 512

    nc = bacc.Bacc(
        "TRN2",
        target_bir_lowering=False,
        debug=False,
        num_devices=NCORES,
        num_swdge_queues=NQUEUES,
    )
    x_d = nc.dram_tensor("x", [SPC, C, TP], dt.bfloat16, kind="ExternalInput")
    mn_d = nc.dram_tensor("maskneg", [SPC, TP], dt.bfloat16, kind="ExternalInput")
    wt_d = nc.dram_tensor("wt", [C, BN], dt.bfloat16, kind="ExternalInput")
    wa_d = nc.dram_tensor("wa", [BN, C], dt.bfloat16, kind="ExternalInput")
    bt_d = nc.dram_tensor("bt", [BN, 1], dt.float32, kind="ExternalInput")
    out_d = nc.dram_tensor("out", [SPC, 2 * C], dt.float32, kind="ExternalOutput")

    if reps is None:
        n_iter, unroll = None, 1
    else:
        unroll = next(k for k in (UNROLL, 8, 4, 2, 1) if reps % k == 0)
        n_iter = reps // unroll

    with tile.TileContext(nc) as tc:
        with (
            tc.tile_pool(name="const", bufs=1) as constp,
            tc.tile_pool(name="xin", bufs=4 * CK) as xp,
            tc.tile_pool(name="esb", bufs=4) as ep,
            tc.tile_pool(name="expm", bufs=4) as xpm,
            tc.tile_pool(name="prod", bufs=DEPTH + 3) as prp,
            tc.tile_pool(name="mneg", bufs=4) as mnp,
            tc.tile_pool(name="stats", bufs=2) as statsp,
            tc.tile_pool(name="tail", bufs=2) as tailp,
            tc.tile_pool(name="ps", bufs=2, space="PSUM") as psp,
            tc.tile_pool(name="pse", bufs=2, space="PSUM") as psep,
        ):
            # ---- constants ------------------------------------------------
            wt_sb = constp.tile([128, CK, BN], dt.bfloat16, tag="wt")
            nc.sync.dma_start(
                out=wt_sb, in_=wt_d.ap().rearrange("(k p) o -> p k o", p=128)
            )
            wa_sb = constp.tile([128, C], dt.bfloat16, tag="wa")
            nc.sync.dma_start(out=wa_sb, in_=wa_d.ap())
            bt_sb = constp.tile([128, 1], dt.float32, tag="bt")
            nc.sync.dma_start(out=bt_sb, in_=bt_d.ap())
            ones_sb = constp.tile([1, 128], dt.bfloat16, tag="ones")
            nc.vector.memset(ones_sb, 1.0)

            def emit_body(u):
                stats = []
                for s in range(SPC):
                    S0 = statsp.tile(
                        [128, CK], dt.float32, tag=f"S0_{s}", name=f"S0_{u}_{s}"
                    )
                    S1 = statsp.tile(
                        [128, CK], dt.float32, tag=f"S1_{s}", name=f"S1_{u}_{s}"
                    )
                    S2 = statsp.tile(
                        [128, CK], dt.float32, tag=f"S2_{s}", name=f"S2_{u}_{s}"
                    )
                    stats.append((S0, S1, S2))

                def tail_stage(s):
                    # mean/std + output DMA for one finished sample
                    S0, S1, S2 = stats[s]
                    r0 = tailp.tile(
                        [128, CK], dt.float32, tag="r0", name=f"r0_{u}_{s}"
                    )
                    nc.vector.reciprocal(out=r0, in_=S0)
                    mean = tailp.tile(
                        [128, CK], dt.float32, tag="mean", name=f"mean_{u}_{s}"
                    )
                    nc.vector.tensor_tensor(out=mean, in0=S1, in1=r0, op=OP.mult)
                    ex2 = tailp.tile(
                        [128, CK], dt.float32, tag="ex2", name=f"ex2_{u}_{s}"
                    )
                    nc.vector.tensor_tensor(out=ex2, in0=S2, in1=r0, op=OP.mult)
                    m2 = tailp.tile(
                        [128, CK], dt.float32, tag="m2", name=f"m2_{u}_{s}"
                    )
                    nc.vector.tensor_tensor(out=m2, in0=mean, in1=mean, op=OP.mult)
                    var = tailp.tile(
                        [128, CK], dt.float32, tag="var", name=f"var_{u}_{s}"
                    )
                    nc.vector.tensor_tensor(out=var, in0=ex2, in1=m2, op=OP.subtract)
                    nc.vector.tensor_scalar(
                        out=var, in0=var, scalar1=1e-9, scalar2=None, op0=OP.max
                    )
                    std = tailp.tile(
                        [128, CK], dt.float32, tag="std", name=f"std_{u}_{s}"
                    )
                    nc.scalar.activation(out=std, in_=var, func=AF.Sqrt)
                    nc.sync.dma_start(
                        out=out_d.ap()[s, 0:C].rearrange("(ck p) -> p ck", p=128),
                        in_=mean,
                    )
                    nc.sync.dma_start(
                        out=out_d.ap()[s, C : 2 * C].rearrange(
                            "(ck p) -> p ck", p=128
                        ),
                        in_=std,
                    )

                # process samples in pairs; the two chunk streams interleave
                # so ACT/DVE always have an independent chunk to work on.
                # The NEXT pair's x DMA and mm1+tanh run during the current
                # pair's chunk stream (mm1 uses its own 1-bank PSUM piece
                # pool so it never contends with the mm2 pa tiles).
                xts = {}
                mnegs = {}
                esbs = {}

                def emit_dma(pair):
                    for s in pair:
                        mneg_sb = mnp.tile(
                            [1, TP], dt.bfloat16, tag="mneg", name=f"mneg_{u}_{s}"
                        )
                        nc.sync.dma_start(out=mneg_sb, in_=mn_d.ap()[s : s + 1, :])
                        mnegs[s] = mneg_sb
                        for k in range(CK):
                            xt = xp.tile(
                                [128, TP], dt.bfloat16, tag="x",
                                name=f"x_{u}_{s}_{k}",
                            )
                            nc.sync.dma_start(
                                out=xt, in_=x_d.ap()[s, k * 128 : (k + 1) * 128, :]
                            )
                            xts[(s, k)] = xt

                def emit_mm1(s):
                    # piecewise mm1 + tanh (one PSUM bank per piece)
                    e_sb = ep.tile(
                        [128, TP], dt.bfloat16, tag="e", name=f"e_{u}_{s}"
                    )
                    for pj, (j0, j1) in enumerate(banks):
                        w = j1 - j0
                        pse = psep.tile(
                            [128, 512], dt.float32, tag="pse",
                            name=f"pse_{u}_{s}_{pj}",
                        )
                        for k in range(CK):
                            nc.tensor.matmul(
                                pse[:, :w],
                                lhsT=wt_sb[:, k, :],
                                rhs=xts[(s, k)][:, j0:j1],
                                start=(k == 0),
                                stop=(k == CK - 1),
                            )
                        nc.scalar.activation(
                            out=e_sb[:, j0:j1],
                            in_=pse[:, :w],
                            func=AF.Tanh,
                            bias=bt_sb,
                            scale=1.0,
                        )
                    esbs[s] = e_sb

                def s2_stage(item):
                    s, c, p1, p2 = item
                    S2 = stats[s][2]
                    junk = prp.tile(
                        [128, TP], dt.bfloat16, tag="junk",
                        name=f"junk_{u}_{s}_{c}",
                    )
                    if S2_MODE[c] == "dve":
                        nc.vector.scalar_tensor_tensor(
                            out=junk,
                            in0=p1,
                            scalar=1.0,
                            in1=xts[(s, c)],
                            op0=OP.mult,
                            op1=OP.mult,
                            accum_out=S2[:, c : c + 1],
                        )
                    else:
                        nc.scalar.activation(
                            out=junk,
                            in_=p2,
                            func=AF.Copy,
                            accum_out=S2[:, c : c + 1],
                        )

                pairs = [[s0, s0 + 1] for s0 in range(0, SPC, 2)]
                emit_dma(pairs[0])
                for s in pairs[0]:
                    emit_mm1(s)

                pending = []
                tail_at = -(-DEPTH // 2)  # prior pair's S2s drained by then
                for pi, pair in enumerate(pairs):
                    for c in range(CK):
                        # prior pair's tails overlap with this pair's work;
                        # by chunk `tail_at` the pending queue has flushed
                        # every S2 belonging to the previous pair
                        if pi > 0 and c == tail_at:
                            tail_stage(pair[0] - 2)
                            tail_stage(pair[0] - 1)
                        for s in pair:
                            S0, S1, S2 = stats[s]
                            e_sb = esbs[s]
                            mneg_sb = mnegs[s]
                            xt = xts[(s, c)]
                            expm = xpm.tile(
                                [128, TP], dt.bfloat16, tag="expm",
                                name=f"expm_{u}_{s}_{c}",
                            )
                            pa_full = psp.tile(
                                [128, PSW], dt.float32, tag="ps",
                                name=f"pa_{u}_{s}_{c}",
                            )
                            pa = pa_full[:, :TP]
                            for (j0, j1) in banks:
                                nc.tensor.matmul(
                                    pa[:, j0:j1],
                                    lhsT=wa_sb[:, c * 128 : (c + 1) * 128],
                                    rhs=e_sb[:, j0:j1],
                                    start=True,
                                    stop=False,
                                )
                            for (j0, j1) in banks:
                                nc.tensor.matmul(
                                    pa[:, j0:j1],
                                    lhsT=ones_sb[:, :],
                                    rhs=mneg_sb[:, j0:j1],
                                    start=False,
                                    stop=True,
                                )
                            nc.scalar.activation(
                                out=expm,
                                in_=pa,
                                func=AF.Exp,
                                accum_out=S0[:, c : c + 1],
                            )
                            # p1 = expm*x with S1 = sum(p1) fused
                            p1 = prp.tile(
                                [128, TP], dt.bfloat16, tag="p1",
                                name=f"p1_{u}_{s}_{c}",
                            )
                            nc.vector.scalar_tensor_tensor(
                                out=p1,
                                in0=expm,
                                scalar=1.0,
                                in1=xt,
                                op0=OP.mult,
                                op1=OP.mult,
                                accum_out=S1[:, c : c + 1],
                            )
                            p2 = None
                            if S2_MODE[c] != "dve":
                                p2 = prp.tile(
                                    [128, TP], dt.bfloat16, tag="p2",
                                    name=f"p2_{u}_{s}_{c}",
                                )
                                eng = (
                                    nc.gpsimd if S2_MODE[c] == "pool" else nc.vector
                                )
                                eng.tensor_tensor(out=p2, in0=p1, in1=xt, op=OP.mult)
                            pending.append((s, c, p1, p2))
                            if len(pending) > DEPTH:
                                s2_stage(pending.pop(0))
                        # prefetch / pre-compute for the next pair
                        if pi + 1 < len(pairs):
                            if c == DMA_C:
                                emit_dma(pairs[pi + 1])
                            elif c == MM1_C[0]:
                                emit_mm1(pairs[pi + 1][0])
                            elif c == MM1_C[1]:
                                emit_mm1(pairs[pi + 1][1])
                for item in pending:
                    s2_stage(item)

                tail_stage(SPC - 2)
                tail_stage(SPC - 1)

            if n_iter is None:
                emit_body(0)
            else:
                with tc.For_i(0, n_iter, 1):
                    for u in range(unroll):
                        emit_body(u)

    nc.compile()
    nc.m = get_hw_module(nc.m)
    return nc


def _get_program(tp):
    key = ("nc", tp)
    if key not in _PROG_CACHE:
        _PROG_CACHE[key] = _build_program(tp=tp)
    return _PROG_CACHE[key]


def _prep_inputs(x, padding_mask, W_tdnn, b_tdnn, W_attn, b_attn):
    """Host-side prep: mask-compact x columns, cast/transpose, build
    per-core input maps."""
    x = np.asarray(x)
    padding_mask = np.asarray(padding_mask)
    keep = ~padding_mask  # (B, T) bool
    counts = keep.sum(axis=1)
    tp = int(counts.max())
    tp = max(256, -(-tp // 128) * 128)  # round up to 128 cols
    _STATE["tp"] = tp

    xc = np.zeros((B, C, tp), dtype=BF16)
    mneg = np.full((B, tp), -1e9, dtype=np.float32)
    for s in range(B):
        n = int(counts[s])
        xc[s, :, :n] = x[s][:, keep[s]].astype(BF16)
        mneg[s, :n] = 0.0
    mneg = mneg.astype(BF16)

    wt = np.ascontiguousarray(W_tdnn.T).astype(BF16)  # (C, BN)
    wa = np.ascontiguousarray(W_attn.T).astype(BF16)  # (BN, C)
    bt = np.ascontiguousarray(b_tdnn.astype(np.float32).reshape(BN, 1))
    in_maps = []
    for i in range(NCORES):
        sl = slice(i * SPC, (i + 1) * SPC)
        in_maps.append(
            {
                "x": np.ascontiguousarray(xc[sl]),
                "maskneg": np.ascontiguousarray(mneg[sl]),
                "wt": wt,
                "wa": wa,
                "bt": bt,
            }
        )
    return in_maps


def kernel(x, padding_mask, W_tdnn, b_tdnn, W_attn, b_attn):
    from concourse.bass_utils import run_bass_kernel_spmd

    in_maps = _prep_inputs(x, padding_mask, W_tdnn, b_tdnn, W_attn, b_attn)
    nc = _get_program(_STATE["tp"])
    res = run_bass_kernel_spmd(nc, in_maps, core_ids=list(range(NCORES)))
    out = np.concatenate([res.results[i]["out"] for i in range(NCORES)], axis=0)
    return out.astype(np.float32)
